# revision 3
# baseline (speedup 1.0000x reference)
"""Distributed Trainium2 kernel for AttentionalPropagation (SuperGlue-style).

Reference computation (B=4, D=256, H=4, N=2048):
    q = Wq x ; k = Wk s ; v = Wv s              (1x1 convs, biases bq/bk/bv)
    prob = softmax(q^T k / sqrt(D))  per (b, h)
    msg  = Wm (v prob^T) + bm
    h1   = W1 [x; msg] + b1
    y    = BN(h1) * gamma + beta ; relu
    out  = W2 y + b2

Sharding: 16 (b, h) pairs, 2 per core across 8 NeuronCores. Only cross-core
dependency is the BatchNorm statistics (4 KB AllReduce).

Algebraic folds (host side):
  scores = x^T (A s)  with A = Wq^T Wk   (bq/bk per-query terms cancel in the
           softmax; the per-key term exp(bq.k/16) is folded into the vT scale)
  v'     = B s        with B = Wm Wv    (Wm applied before attention mixing;
           Wm bv + bm folds through W1 into the BN shift)
  h1     = W1 [x; msg'] stored with NO biases; all of b1/bm/bv and the BN
           mean/beta collapse into a single per-channel shift t applied in
           pass 2:  out = (W2 diag(scl)) relu(h1 - mu + beta/scl) + b2.

Precision: scores and msg GEMMs run fp8e4 (DoubleRow, 2x PE rate) -- verified
numerically harmless because msg << x in h1. W1/W2 GEMMs stay bf16.

Engine balance: TensorE GEMMs; ScalarE exp (batched [128,2048] from a 4-bank
PSUM tile) + W2 evac; VectorE PSUM evacuations + bn_stats; Pool/GPSIMD the
BN-relu apply and weight rescale (SBUF-only engine).
"""

import sys
from functools import partial

import numpy as np

sys.path.insert(0, "/opt/trn_rl_repo")

import concourse.bass as bass
import concourse.bacc as bacc
import concourse.tile as tile
from concourse import mybir
from concourse.bass_utils import run_bass_kernel_spmd

import ml_dtypes

BF16 = ml_dtypes.bfloat16
F8 = ml_dtypes.float8_e4m3

B, D, H, N = 4, 256, 4, 2048
EPS = 1e-5
P = 128
NCORES = 8
PAIRS = (B * H) // NCORES  # 2 per core
CT = D // P       # 2 k-tiles for D
CT2 = 2 * D // P  # 4 k-tiles for 2D
MT = N // P       # 16 key tiles
NCH = 4           # 512-wide n chunks
CHUNK = N // NCH

SA = 64.0    # A scale (A8 = SA * A)
SB = 128.0   # B scale
SC_EXP = 1.0 / (16.0 * SA)

AF = mybir.ActivationFunctionType
ALU = mybir.AluOpType
DR = mybir.MatmulPerfMode.DoubleRow
f32 = mybir.dt.float32
bf16 = mybir.dt.bfloat16
fp8 = mybir.dt.float8e4

_CACHE = {}


def build_bass() -> bass.Bass:
    nc = bacc.Bacc("TRN2", num_devices=NCORES)

    x8d = nc.dram_tensor("x8", [PAIRS, P, CT, N], fp8, kind="ExternalInput")
    x16d = nc.dram_tensor("x16", [PAIRS, P, CT, N], bf16, kind="ExternalInput")
    s8d = nc.dram_tensor("s8", [PAIRS, P, CT, N], fp8, kind="ExternalInput")
    a8d = nc.dram_tensor("a8", [P, CT, D], fp8, kind="ExternalInput")
    b8d = nc.dram_tensor("b8", [P, CT, D], fp8, kind="ExternalInput")
    w1d = nc.dram_tensor("w1", [P, CT2, 2 * D], bf16, kind="ExternalInput")
    w2d = nc.dram_tensor("w2", [P, CT2, D], bf16, kind="ExternalInput")
    vecd = nc.dram_tensor("vec", [P, 80], f32, kind="ExternalInput")
    outd = nc.dram_tensor("out", [PAIRS, CT, P, N], bf16, kind="ExternalOutput")

    cc_in = nc.dram_tensor("cc_in", [P, 8], f32)
    cc_out = nc.dram_tensor("cc_out", [P, 8], f32, addr_space="Shared")
    cw_in = nc.dram_tensor("cw_in", [1, 8], f32)
    cw_out = nc.dram_tensor("cw_out", [1, 8], f32, addr_space="Shared")

    with tile.TileContext(nc) as tc:
        with (
            tc.tile_pool(name="consts", bufs=1) as consts,
            tc.tile_pool(name="persist", bufs=1) as persist,
            tc.tile_pool(name="pairbuf", bufs=2) as pairbuf,
            tc.tile_pool(name="work", bufs=2) as work,
            tc.tile_pool(name="pbig", bufs=1, space="PSUM") as pbig,
            tc.tile_pool(name="pmmw", bufs=1, space="PSUM") as pmmw,
            tc.tile_pool(name="pmsg", bufs=2, space="PSUM") as pmsg,
        ):
            # ---- weight/const loads (gpsimd SWDGE queue) ----
            a8s = consts.tile([P, CT, D], fp8, tag="a8s")
            b8s = consts.tile([P, CT, D], fp8, tag="b8s")
            w1s = consts.tile([P, CT2, 2 * D], bf16, tag="w1s")
            w2s = consts.tile([P, CT2, D], bf16, tag="w2s")
            vec = consts.tile([P, 80], f32, tag="vec")
            for t_, d_ in ((a8s, a8d), (b8s, b8d), (w1s, w1d), (w2s, w2d), (vec, vecd)):
                nc.gpsimd.dma_start(out=t_[:], in_=d_[:])
            expd4 = [vec[:, 0:16], vec[:, 16:32]]
            expdD = [vec[:, 32:48], vec[:, 48:64]]
            b2col = vec[:, 64:66]
            gamma4 = vec[:, 66:70]
            beta4 = vec[:, 70:74]
            eps_t = vec[:, 74:75]

            # ---- per-pair inputs, all loaded up front ----
            # pair 0 on the sync queue, pair 1 on the scalar HWDGE queue.
            x8t, x16t, s8t = [], [], []
            for p in range(PAIRS):
                q = nc.sync if p == 0 else nc.scalar
                x8_ = persist.tile([P, CT, N], fp8, tag=f"x8_{p}")
                s8_ = persist.tile([P, CT, N], fp8, tag=f"s8_{p}")
                x16_ = persist.tile([P, CT, N], bf16, tag=f"x16_{p}")
                q.dma_start(out=s8_[:], in_=s8d[p])
                q.dma_start(out=x8_[:], in_=x8d[p])
                q.dma_start(out=x16_[:], in_=x16d[p])
                x8t.append(x8_)
                x16t.append(x16_)
                s8t.append(s8_)

            # ---- warmups: ACT tables (ln+exp set), PE ramp, collective ----
            warm = persist.tile([P, 1], f32, tag="warm")
            nc.vector.memset(warm, 1.0)
            nc.scalar.activation(warm, warm, AF.Ln)
            nc.scalar.activation(warm, warm, AF.Exp)

            pe_w = persist.tile([P, CHUNK], bf16, tag="pe_w")
            nc.vector.memset(pe_w, 0.0)
            for _ in range(8):
                pw = pmmw.tile([P, 2, CHUNK], f32, tag="mmw", name="mmw")
                nc.tensor.matmul(pw[:, 0, :], pe_w[:, 0:P], pe_w, start=True, stop=True)

            nc.gpsimd.collective_compute(
                "AllReduce", ALU.add,
                replica_groups=[list(range(NCORES))],
                ins=[cw_in[:].opt()], outs=[cw_out[:].opt()],
            )

            # ---- persistent state ----
            h1 = [persist.tile([P, CT2, N], bf16, tag=f"h1_{p}", name=f"h1_{p}") for p in range(PAIRS)]
            # bn_stats slots: [pair, m, chunk, 6]
            bnbuf = persist.tile([P, PAIRS, CT2, NCH, 6], bf16, tag="bnbuf")
            big = pbig.tile([P, N], f32, tag="big", name="big")

            as8t, vT8t, e8t = [None] * PAIRS, [None] * PAIRS, [None] * PAIRS
            msgTt, msg2t = [None] * PAIRS, [None] * PAIRS

            def proj_tasks(p):
                """as = A s and vT = (B s)^T projections for pair p (fp8 DR)."""
                as8 = pairbuf.tile([P, CT, N], fp8, tag="as8", name="as8")
                vT8 = pairbuf.tile([P, MT, D + 1], fp8, tag="vT8", name="vT8")
                as8t[p], vT8t[p] = as8, vT8
                tasks = []

                def as_chunk(m, jp):
                    ps = pmmw.tile([P, 2, CHUNK], f32, tag="mmw", name="mmw")
                    for jj in range(2):
                        j = jp * 2 + jj
                        nc.tensor.matmul(
                            ps[:, jj, :],
                            a8s[:, :, m * P:(m + 1) * P],
                            s8t[p][:, :, j * CHUNK:(j + 1) * CHUNK],
                            start=True, stop=True, perf_mode=DR,
                        )
                    nc.vector.tensor_copy(
                        as8[:, m, jp * 2 * CHUNK:(jp + 1) * 2 * CHUNK], ps[:]
                    )

                def vt_chunk(tp):
                    ps = pmmw.tile([P, 2, CHUNK], f32, tag="mmw", name="mmw")
                    for tt in range(2):
                        t = tp * 2 + tt
                        nc.tensor.matmul(
                            ps[:, tt, 0:D],
                            s8t[p][:, :, t * P:(t + 1) * P],
                            b8s[:],
                            start=True, stop=True, perf_mode=DR,
                        )
                    for tt in range(2):
                        t = tp * 2 + tt
                        nc.vector.tensor_scalar_mul(
                            vT8[:, t, 0:D], ps[:, tt, 0:D], expd4[p][:, t:t + 1]
                        )

                for m in range(CT):
                    for jp in range(2):
                        tasks.append(partial(as_chunk, m, jp))
                for tp in range(MT // 2):
                    tasks.append(partial(vt_chunk, tp))
                # denominator column: 32 * expd per key
                tasks.append(lambda: nc.vector.tensor_copy(vT8[:, :, D], expdD[p][:]))
                return tasks

            def scores_exp(p, fills):
                """fp8 scores + batched exp into e8[p]; weave fill tasks."""
                e8 = pairbuf.tile([P, MT, N], fp8, tag="e8", name="e8")
                e8t[p] = e8
                fi = 0
                for t in range(MT):
                    for j in range(NCH):
                        nc.tensor.matmul(
                            big[:, j * CHUNK:(j + 1) * CHUNK],
                            as8t[p][:, :, t * P:(t + 1) * P],
                            x8t[p][:, :, j * CHUNK:(j + 1) * CHUNK],
                            start=True, stop=True, perf_mode=DR,
                        )
                    nc.scalar.activation(e8[:, t, :], big[:], AF.Exp, scale=SC_EXP)
                    tgt = (t + 1) * len(fills) // MT
                    while fi < tgt:
                        fills[fi]()
                        fi += 1
                while fi < len(fills):
                    fills[fi]()
                    fi += 1

            def msg_tasks(p):
                """msg GEMM (fp8 DR over key tiles) + normalize + transpose."""
                msgT = work.tile([P, MT, D], bf16, tag="msgT", name="msgT")
                msg2 = work.tile([P, CT, N], bf16, tag="msg2", name="msg2")
                msgTt[p], msg2t[p] = msgT, msg2
                e8, vT8 = e8t[p], vT8t[p]
                tasks = []

                def msg_sub(nsub):
                    u = nsub % NCH
                    j = nsub // NCH
                    ps = pmsg.tile([P, D + 1], f32, tag="msgp", name="msgp")
                    for tp in range(MT // 2):
                        nc.tensor.matmul(
                            ps,
                            e8[:, 2 * tp:2 * tp + 2, nsub * P:(nsub + 1) * P],
                            vT8[:, 2 * tp:2 * tp + 2, :],
                            start=(tp == 0), stop=(tp == MT // 2 - 1),
                            perf_mode=DR,
                        )
                    rec = work.tile([P, 1], f32, tag="rec", bufs=2, name="rec")
                    nc.vector.reciprocal(rec, ps[:, D:D + 1])
                    nc.vector.tensor_scalar_mul(msgT[:, nsub, :], ps[:, 0:D], rec)
                    nc.sync.dma_start_transpose(
                        out=msg2[:, :, nsub * P:(nsub + 1) * P],
                        in_=msgT[:, nsub, :],
                    )

                for nsub in range(MT):
                    tasks.append(partial(msg_sub, nsub))
                return tasks

            def w1_tasks(p):
                """h1 = W1 [x; msg2] (bf16), no bias; bn_stats per chunk."""
                tasks = []

                def w1_chunk(m, jp):
                    ps = pmmw.tile([P, 2, CHUNK], f32, tag="mmw", name="mmw")
                    for jj in range(2):
                        j = jp * 2 + jj
                        sl = slice(j * CHUNK, (j + 1) * CHUNK)
                        for k in range(CT2):
                            rhs = x16t[p][:, k, sl] if k < CT else msg2t[p][:, k - CT, sl]
                            nc.tensor.matmul(
                                ps[:, jj, :],
                                w1s[:, k, m * P:(m + 1) * P],
                                rhs,
                                start=(k == 0), stop=(k == CT2 - 1),
                            )
                    osl = slice(jp * 2 * CHUNK, (jp + 1) * 2 * CHUNK)
                    nc.vector.tensor_copy(h1[p][:, m, osl], ps[:])
                    for jj in range(2):
                        j = jp * 2 + jj
                        nc.vector.bn_stats(
                            bnbuf[:, p, m, j, :],
                            h1[p][:, m, j * CHUNK:(j + 1) * CHUNK],
                        )

                for m in range(CT2):
                    for jp in range(2):
                        tasks.append(partial(w1_chunk, m, jp))
                return tasks

            # ================= pass 1 =================
            for t_ in proj_tasks(0):
                t_()
            scores_exp(0, proj_tasks(1))
            scores_exp(1, msg_tasks(0) + w1_tasks(0))
            for t_ in msg_tasks(1) + w1_tasks(1):
                t_()

            # ================= BN statistics =================
            stats2 = persist.tile([P, CT2, 2], f32, tag="stats2")
            for m in range(CT2):
                nc.vector.bn_aggr(stats2[:, m, :], bnbuf[:, :, m, :, :])
            cnt_core = float(PAIRS * N)
            cnt_all = float(B * H * N)
            stats_l = persist.tile([P, 2 * CT2], f32, tag="stats_l")
            tmp4 = persist.tile([P, CT2], f32, tag="tmp4")
            nc.vector.tensor_scalar_mul(stats_l[:, 0:CT2], stats2[:, :, 0], cnt_core)
            nc.vector.tensor_mul(tmp4, stats2[:, :, 0], stats2[:, :, 0])
            nc.vector.tensor_add(tmp4, stats2[:, :, 1], tmp4)
            nc.vector.tensor_scalar_mul(stats_l[:, CT2:], tmp4, cnt_core)
            nc.sync.dma_start(out=cc_in[:], in_=stats_l[:])
            # re-warm the ln/exp tables while the collective runs
            nc.scalar.activation(warm, warm, AF.Ln)
            nc.scalar.activation(warm, warm, AF.Exp)
            nc.gpsimd.collective_compute(
                "AllReduce", ALU.add,
                replica_groups=[list(range(NCORES))],
                ins=[cc_in[:].opt()], outs=[cc_out[:].opt()],
            )
            stats_g = persist.tile([P, 2 * CT2], f32, tag="stats_g")
            nc.sync.dma_start(out=stats_g[:], in_=cc_out[:])

            mom = persist.tile([P, 2 * CT2], f32, tag="mom")
            nc.vector.tensor_scalar_mul(mom, stats_g, 1.0 / cnt_all)
            var = persist.tile([P, CT2], f32, tag="var")
            nc.vector.tensor_mul(var, mom[:, 0:CT2], mom[:, 0:CT2])
            nc.vector.tensor_sub(var, mom[:, CT2:], var)
            # rsqrt = exp(-0.5 ln(var+eps)); same act table set as the exp
            lnv = persist.tile([P, CT2], f32, tag="lnv")
            nc.scalar.activation(lnv, var, AF.Ln, bias=eps_t)
            inv = persist.tile([P, CT2], f32, tag="inv")
            nc.scalar.activation(inv, lnv, AF.Exp, scale=-0.5)
            scl4 = persist.tile([P, CT2], f32, tag="scl4")
            nc.vector.tensor_mul(scl4, gamma4, inv)
            rscl = persist.tile([P, CT2], f32, tag="rscl")
            nc.vector.reciprocal(rscl, scl4)
            t4 = persist.tile([P, CT2], f32, tag="t4")
            nc.vector.tensor_mul(t4, beta4, rscl)
            nc.vector.tensor_sub(t4, t4, mom[:, 0:CT2])
            # W2' = W2 * scl (per input channel = partition x ktile)
            w2x = persist.tile([P, CT2, D], bf16, tag="w2x")
            for k in range(CT2):
                nc.gpsimd.tensor_scalar_mul(w2x[:, k, :], w2s[:, k, :], scl4[:, k:k + 1])

            # ================= pass 2 =================
            for p in range(PAIRS):
                for j in range(NCH):
                    sl = slice(j * CHUNK, (j + 1) * CHUNK)
                    h1n = work.tile([P, CT2, CHUNK], bf16, tag="h1n", name="h1n")
                    for m in range(CT2):
                        eng = nc.vector if m < 2 else nc.gpsimd
                        eng.tensor_scalar(
                            h1n[:, m, :], h1[p][:, m, sl],
                            t4[:, m:m + 1], 0.0,
                            op0=ALU.add, op1=ALU.max,
                        )
                    ps = pmmw.tile([P, 2, CHUNK], f32, tag="mmw", name="mmw")
                    for c in range(CT):
                        for k in range(CT2):
                            nc.tensor.matmul(
                                ps[:, c, :],
                                w2x[:, k, c * P:(c + 1) * P],
                                h1n[:, k, :],
                                start=(k == 0), stop=(k == CT2 - 1),
                            )
                    ob = work.tile([P, CT, CHUNK], bf16, tag="ob", name="ob")
                    for c in range(CT):
                        nc.scalar.activation(
                            ob[:, c, :], ps[:, c, :], AF.Identity,
                            bias=b2col[:, c:c + 1],
                        )
                    for c in range(CT):
                        q = nc.sync if (j + c) % 2 == 0 else nc.scalar
                        q.dma_start(out=outd[p, c, :, sl], in_=ob[:, c, :])

    nc.finalize()
    return nc


def _get_nc():
    if "nc" not in _CACHE:
        _CACHE["nc"] = build_bass()
    return _CACHE["nc"]


def _prep_inputs(inputs):
    x = np.asarray(inputs["x"], np.float32)
    source = np.asarray(inputs["source"], np.float32)
    Wq = np.asarray(inputs["Wq"], np.float32)
    Wk = np.asarray(inputs["Wk"], np.float32)
    Wv = np.asarray(inputs["Wv"], np.float32)
    Wm = np.asarray(inputs["Wm"], np.float32)
    W1 = np.asarray(inputs["W1"], np.float32)
    W2 = np.asarray(inputs["W2"], np.float32)
    bq = np.asarray(inputs["bq"], np.float32)
    bk = np.asarray(inputs["bk"], np.float32)

    def to_pairs(a, dt):
        a = a.transpose(0, 2, 1, 3).reshape(B * H, CT, P, N)
        a = np.ascontiguousarray(a.transpose(0, 2, 1, 3))
        if dt is F8:
            a = np.clip(a, -240, 240)
        return a.astype(dt)

    def lhsT(w, dt, scale=1.0):
        wT = np.ascontiguousarray(w.T * scale)
        cin, cout = wT.shape
        a = wT.reshape(cin // P, P, cout).transpose(1, 0, 2)
        a = np.ascontiguousarray(a)
        if dt is F8:
            a = np.clip(a, -240, 240)
        return a.astype(dt)

    def vcol(b):
        return np.asarray(b, np.float32).reshape(-1, P).T

    A = Wq.T @ Wk
    Bm = Wm @ Wv

    # per-key softmax bias term exp((bq . k_m)/16) (zero bq -> ones)
    if np.any(bq):
        kfull = np.einsum("oc,bchn->bohn", Wk, source) + bk[None, :, None, None]
        dm = np.einsum("c,bchn->bhn", bq, kfull) / 16.0
        expd = np.exp(dm).reshape(B * H, MT, P).transpose(0, 2, 1)  # [pair, P, MT]
    else:
        expd = np.ones((B * H, P, MT), np.float32)

    vecs_core = []
    for i in range(NCORES):
        vec = np.zeros((P, 80), np.float32)
        for p in range(PAIRS):
            g = i * PAIRS + p
            vec[:, 16 * p:16 * (p + 1)] = 0.25 * expd[g]
            vec[:, 32 + 16 * p:32 + 16 * (p + 1)] = 32.0 * expd[g]
        vec[:, 64:66] = vcol(inputs["b2"])
        vec[:, 66:70] = vcol(inputs["gamma"])
        vec[:, 70:74] = vcol(inputs["beta"])
        vec[:, 74] = EPS
        vecs_core.append(vec)

    x8 = to_pairs(x, F8)
    x16 = to_pairs(x, BF16)
    s8 = to_pairs(source, F8)

    common = {
        "a8": lhsT(A, F8, SA),
        "b8": lhsT(Bm, F8, SB),
        "w1": lhsT(W1, BF16),
        "w2": lhsT(W2, BF16),
    }
    in_maps = []
    for i in range(NCORES):
        m = dict(common)
        m["vec"] = vecs_core[i]
        m["x8"] = np.ascontiguousarray(x8[i * PAIRS:(i + 1) * PAIRS])
        m["x16"] = np.ascontiguousarray(x16[i * PAIRS:(i + 1) * PAIRS])
        m["s8"] = np.ascontiguousarray(s8[i * PAIRS:(i + 1) * PAIRS])
        in_maps.append(m)
    return in_maps


def run_on_hw(inputs, trace=False, **kw):
    nc = _get_nc()
    in_maps = _prep_inputs(inputs)
    res = run_bass_kernel_spmd(
        nc, in_maps, core_ids=list(range(NCORES)), trace=trace, **kw
    )
    outs = res.results
    full = np.empty((B, H, D, N), np.float32)
    for i in range(NCORES):
        o = np.asarray(outs[i]["out"]).astype(np.float32).reshape(PAIRS, D, N)
        for jp in range(PAIRS):
            g = i * PAIRS + jp
            full[g // H, g % H] = o[jp]
    return full.transpose(0, 2, 1, 3), res


def kernel(**inputs) -> np.ndarray:
    out, _ = run_on_hw(inputs, trace=False)
    return out


# revision 9
# speedup vs baseline: 1.3277x; 1.3277x over previous
"""Distributed Trainium2 kernel for AttentionalPropagation (SuperGlue-style).

Reference computation (B=4, D=256, H=4, N=2048):
    q = Wq x ; k = Wk s ; v = Wv s              (1x1 convs, biases bq/bk/bv)
    prob = softmax(q^T k / sqrt(D))  per (b, h)
    msg  = Wm (v prob^T) + bm
    h1   = W1 [x; msg] + b1
    y    = BN(h1) * gamma + beta ; relu
    out  = W2 y + b2

Sharding: 16 (b, h) pairs, 2 per core across 8 NeuronCores. Only cross-core
dependency is the BatchNorm statistics (4 KB AllReduce).

Algebraic folds (host side):
  scores = x^T (A s)  with A = Wq^T Wk   (bq/bk per-query terms cancel in the
           softmax; the per-key term exp(bq.k/16) is folded into the vT scale)
  v'     = B s        with B = Wm Wv    (Wm applied before attention mixing;
           Wm bv + bm folds through W1 into the BN shift)
  h1     = W1 [x; msg'] stored with NO biases; all of b1/bm/bv and the BN
           mean/beta collapse into a single per-channel shift t applied in
           pass 2:  out = (W2 diag(scl)) relu(h1 - mu + beta/scl) + b2.

Precision: scores and msg GEMMs run fp8e4 (DoubleRow, 2x PE rate) -- verified
numerically harmless because msg << x in h1. W1/W2 GEMMs stay bf16.

Engine balance: TensorE GEMMs; ScalarE exp (batched [128,2048] from a 4-bank
PSUM tile) + W2 evac; VectorE PSUM evacuations + bn_stats; Pool/GPSIMD the
BN-relu apply and weight rescale (SBUF-only engine).
"""

import sys
from functools import partial

import numpy as np

sys.path.insert(0, "/opt/trn_rl_repo")

import concourse.bass as bass
import concourse.bacc as bacc
import concourse.tile as tile
from concourse import mybir
from concourse.bass_utils import run_bass_kernel_spmd

import ml_dtypes

BF16 = ml_dtypes.bfloat16
F8 = ml_dtypes.float8_e4m3

B, D, H, N = 4, 256, 4, 2048
EPS = 1e-5
P = 128
NCORES = 8
PAIRS = (B * H) // NCORES  # 2 per core
CT = D // P       # 2 k-tiles for D
CT2 = 2 * D // P  # 4 k-tiles for 2D
MT = N // P       # 16 key tiles
NCH = 4           # 512-wide n chunks
CHUNK = N // NCH

SA = 64.0    # A scale (A8 = SA * A)
SB = 128.0   # B scale
SC_EXP = 1.0 / (16.0 * SA)

AF = mybir.ActivationFunctionType
ALU = mybir.AluOpType
DR = mybir.MatmulPerfMode.DoubleRow
f32 = mybir.dt.float32
bf16 = mybir.dt.bfloat16
fp8 = mybir.dt.float8e4

_CACHE = {}


def build_bass() -> bass.Bass:
    nc = bacc.Bacc("TRN2", num_devices=NCORES)

    x8d = nc.dram_tensor("x8", [PAIRS, P, CT, N], fp8, kind="ExternalInput")
    x16d = nc.dram_tensor("x16", [PAIRS, P, CT, N], bf16, kind="ExternalInput")
    s8d = nc.dram_tensor("s8", [PAIRS, P, CT, N], fp8, kind="ExternalInput")
    a8d = nc.dram_tensor("a8", [P, CT, D], fp8, kind="ExternalInput")
    b8d = nc.dram_tensor("b8", [P, CT, D], fp8, kind="ExternalInput")
    w1d = nc.dram_tensor("w1", [P, CT2, 2 * D], bf16, kind="ExternalInput")
    w2d = nc.dram_tensor("w2", [P, CT2, D], bf16, kind="ExternalInput")
    vecd = nc.dram_tensor("vec", [P, 80], f32, kind="ExternalInput")
    outd = nc.dram_tensor("out", [PAIRS, CT, P, N], bf16, kind="ExternalOutput")

    cc_in = nc.dram_tensor("cc_in", [P, 8], f32)
    cc_out = nc.dram_tensor("cc_out", [P, 8], f32, addr_space="Shared")
    cw_in = nc.dram_tensor("cw_in", [1, 8], f32)
    cw_out = nc.dram_tensor("cw_out", [1, 8], f32, addr_space="Shared")

    with tile.TileContext(nc) as tc:
        with (
            tc.tile_pool(name="consts", bufs=1) as consts,
            tc.tile_pool(name="persist", bufs=1) as persist,
            tc.tile_pool(name="pairbuf", bufs=2) as pairbuf,
            tc.tile_pool(name="work", bufs=2) as work,
            tc.tile_pool(name="pbig", bufs=2, space="PSUM") as pbig,
            tc.tile_pool(name="pmmw", bufs=1, space="PSUM") as pmmw,
            tc.tile_pool(name="pmsg", bufs=2, space="PSUM") as pmsg,
        ):
            # ---- weight/const loads (gpsimd SWDGE queue) ----
            a8s = consts.tile([P, CT, D], fp8, tag="a8s")
            b8s = consts.tile([P, CT, D], fp8, tag="b8s")
            w1s = consts.tile([P, CT2, 2 * D], bf16, tag="w1s")
            w2s = consts.tile([P, CT2, D], bf16, tag="w2s")
            vec = consts.tile([P, 80], f32, tag="vec")
            for t_, d_ in ((a8s, a8d), (b8s, b8d), (w1s, w1d), (w2s, w2d), (vec, vecd)):
                nc.gpsimd.dma_start(out=t_[:], in_=d_[:])
            expd4 = [vec[:, 0:16], vec[:, 16:32]]
            expdD = [vec[:, 32:48], vec[:, 48:64]]
            b2col = vec[:, 64:66]
            gamma4 = vec[:, 66:70]
            beta4 = vec[:, 70:74]
            eps_t = vec[:, 74:75]

            # ---- per-pair inputs, all loaded up front ----
            # s8(0) is needed first (projections): split it across both HWDGE
            # queues; everything else alternates so neither queue serializes.
            x8t, x16t, s8t = [], [], []
            for p in range(PAIRS):
                x8_ = persist.tile([P, CT, N], fp8, tag=f"x8_{p}")
                s8_ = persist.tile([P, CT, N], fp8, tag=f"s8_{p}")
                x16_ = persist.tile([P, CT, N], bf16, tag=f"x16_{p}")
                x8t.append(x8_)
                x16t.append(x16_)
                s8t.append(s8_)
            hn = N // 2
            nc.sync.dma_start(out=s8t[0][:, :, 0:hn], in_=s8d[0, :, :, 0:hn])
            nc.scalar.dma_start(out=s8t[0][:, :, hn:], in_=s8d[0, :, :, hn:])
            nc.sync.dma_start(out=x8t[0][:], in_=x8d[0])
            nc.scalar.dma_start(out=s8t[1][:], in_=s8d[1])
            nc.sync.dma_start(out=x8t[1][:], in_=x8d[1])
            nc.scalar.dma_start(out=x16t[0][:], in_=x16d[0])
            nc.sync.dma_start(out=x16t[1][:], in_=x16d[1])

            # ---- warmups: ACT tables (ln+exp set), PE ramp, collective ----
            warm = persist.tile([P, 1], f32, tag="warm")
            nc.vector.memset(warm, 1.0)
            nc.scalar.activation(warm, warm, AF.Ln)
            nc.scalar.activation(warm, warm, AF.Exp)

            pe_w = persist.tile([P, CHUNK], bf16, tag="pe_w")
            nc.vector.memset(pe_w, 0.0)
            for _ in range(8):
                pw = pmmw.tile([P, 2, CHUNK], f32, tag="mmw", name="mmw")
                nc.tensor.matmul(pw[:, 0, :], pe_w[:, 0:P], pe_w, start=True, stop=True)

            nc.gpsimd.collective_compute(
                "AllReduce", ALU.add,
                replica_groups=[list(range(NCORES))],
                ins=[cw_in[:].opt()], outs=[cw_out[:].opt()],
            )

            # ---- persistent state ----
            h1 = [persist.tile([P, CT2, N], bf16, tag=f"h1_{p}", name=f"h1_{p}") for p in range(PAIRS)]
            # bn_stats slots: [pair, m, chunk, 6]
            bnbuf = persist.tile([P, PAIRS, CT2, NCH, 6], bf16, tag="bnbuf")

            as8t, vT8t, e8t = [None] * PAIRS, [None] * PAIRS, [None] * PAIRS
            msgTt, msg2t = [None] * PAIRS, [None] * PAIRS

            def proj_tasks(p):
                """as = A s and vT = (B s)^T projections for pair p (fp8 DR)."""
                as8 = pairbuf.tile([P, CT, N], fp8, tag="as8", name="as8")
                vT8 = pairbuf.tile([P, MT, D + 1], fp8, tag="vT8", name="vT8")
                as8t[p], vT8t[p] = as8, vT8
                tasks = []

                def as_chunk(m, jp):
                    ps = pmmw.tile([P, 2, CHUNK], f32, tag="mmw", name="mmw")
                    for jj in range(2):
                        j = jp * 2 + jj
                        nc.tensor.matmul(
                            ps[:, jj, :],
                            a8s[:, :, m * P:(m + 1) * P],
                            s8t[p][:, :, j * CHUNK:(j + 1) * CHUNK],
                            start=True, stop=True, perf_mode=DR,
                        )
                    nc.vector.tensor_copy(
                        as8[:, m, jp * 2 * CHUNK:(jp + 1) * 2 * CHUNK], ps[:]
                    )

                def vt_chunk(tp):
                    ps = pmmw.tile([P, 2, CHUNK], f32, tag="mmw", name="mmw")
                    for tt in range(2):
                        t = tp * 2 + tt
                        nc.tensor.matmul(
                            ps[:, tt, 0:D],
                            s8t[p][:, :, t * P:(t + 1) * P],
                            b8s[:],
                            start=True, stop=True, perf_mode=DR,
                        )
                    for tt in range(2):
                        t = tp * 2 + tt
                        nc.vector.tensor_scalar_mul(
                            vT8[:, t, 0:D], ps[:, tt, 0:D], expd4[p][:, t:t + 1]
                        )

                for m in range(CT):
                    for jp in range(2):
                        tasks.append(partial(as_chunk, m, jp))
                for tp in range(MT // 2):
                    tasks.append(partial(vt_chunk, tp))
                # denominator column: 32 * expd per key
                tasks.append(lambda: nc.vector.tensor_copy(vT8[:, :, D], expdD[p][:]))
                return tasks

            def scores_exp(p, fills):
                """fp8 scores + batched exp into e8[p]; weave fill tasks."""
                e8 = pairbuf.tile([P, MT, N], fp8, tag="e8", name="e8")
                e8t[p] = e8
                fi = 0
                for t in range(MT):
                    for hh in range(2):
                        big = pbig.tile([P, N // 2], f32, tag="big", name="big")
                        for jj in range(2):
                            j = hh * 2 + jj
                            nc.tensor.matmul(
                                big[:, jj * CHUNK:(jj + 1) * CHUNK],
                                as8t[p][:, :, t * P:(t + 1) * P],
                                x8t[p][:, :, j * CHUNK:(j + 1) * CHUNK],
                                start=True, stop=True, perf_mode=DR,
                            )
                        nc.scalar.activation(
                            e8[:, t, hh * 1024:(hh + 1) * 1024], big[:],
                            AF.Exp, scale=SC_EXP,
                        )
                    tgt = (t + 1) * len(fills) // MT
                    while fi < tgt:
                        fills[fi]()
                        fi += 1
                while fi < len(fills):
                    fills[fi]()
                    fi += 1

            def msg_tasks(p):
                """msg GEMM (fp8 DR over key tiles) + normalize + transpose."""
                msgT = work.tile([P, MT, D], bf16, tag="msgT", name="msgT")
                msg2 = work.tile([P, CT, N], bf16, tag="msg2", name="msg2")
                msgTt[p], msg2t[p] = msgT, msg2
                e8, vT8 = e8t[p], vT8t[p]
                tasks = []

                def msg_sub(nsub):
                    u = nsub % NCH
                    j = nsub // NCH
                    ps = pmsg.tile([P, D + 1], f32, tag="msgp", name="msgp")
                    for tp in range(MT // 2):
                        nc.tensor.matmul(
                            ps,
                            e8[:, 2 * tp:2 * tp + 2, nsub * P:(nsub + 1) * P],
                            vT8[:, 2 * tp:2 * tp + 2, :],
                            start=(tp == 0), stop=(tp == MT // 2 - 1),
                            perf_mode=DR,
                        )
                    rec = work.tile([P, 1], f32, tag="rec", bufs=2, name="rec")
                    nc.vector.reciprocal(rec, ps[:, D:D + 1])
                    nc.vector.tensor_scalar_mul(msgT[:, nsub, :], ps[:, 0:D], rec)
                    nc.sync.dma_start_transpose(
                        out=msg2[:, :, nsub * P:(nsub + 1) * P],
                        in_=msgT[:, nsub, :],
                    )

                for nsub in range(MT):
                    tasks.append(partial(msg_sub, nsub))
                return tasks

            def w1_tasks(p):
                """h1 = W1 [x; msg2] (bf16), no bias; bn_stats per chunk."""
                tasks = []

                def w1_chunk(m, jp):
                    ps = pmmw.tile([P, 2, CHUNK], f32, tag="mmw", name="mmw")
                    for jj in range(2):
                        j = jp * 2 + jj
                        sl = slice(j * CHUNK, (j + 1) * CHUNK)
                        for k in range(CT2):
                            rhs = x16t[p][:, k, sl] if k < CT else msg2t[p][:, k - CT, sl]
                            nc.tensor.matmul(
                                ps[:, jj, :],
                                w1s[:, k, m * P:(m + 1) * P],
                                rhs,
                                start=(k == 0), stop=(k == CT2 - 1),
                            )
                    osl = slice(jp * 2 * CHUNK, (jp + 1) * 2 * CHUNK)
                    nc.vector.tensor_copy(h1[p][:, m, osl], ps[:])
                    for jj in range(2):
                        j = jp * 2 + jj
                        nc.vector.bn_stats(
                            bnbuf[:, p, m, j, :],
                            h1[p][:, m, j * CHUNK:(j + 1) * CHUNK],
                        )

                for m in range(CT2):
                    for jp in range(2):
                        tasks.append(partial(w1_chunk, m, jp))
                return tasks

            # ================= pass 1 =================
            for t_ in proj_tasks(0):
                t_()
            scores_exp(0, proj_tasks(1))
            scores_exp(1, msg_tasks(0) + w1_tasks(0))
            for t_ in msg_tasks(1) + w1_tasks(1):
                t_()

            # ================= BN statistics =================
            stats2 = persist.tile([P, CT2, 2], f32, tag="stats2")
            for m in range(CT2):
                nc.vector.bn_aggr(stats2[:, m, :], bnbuf[:, :, m, :, :])
            cnt_core = float(PAIRS * N)
            cnt_all = float(B * H * N)
            stats_l = persist.tile([P, 2 * CT2], f32, tag="stats_l")
            tmp4 = persist.tile([P, CT2], f32, tag="tmp4")
            nc.vector.tensor_scalar_mul(stats_l[:, 0:CT2], stats2[:, :, 0], cnt_core)
            nc.vector.tensor_mul(tmp4, stats2[:, :, 0], stats2[:, :, 0])
            nc.vector.tensor_add(tmp4, stats2[:, :, 1], tmp4)
            nc.vector.tensor_scalar_mul(stats_l[:, CT2:], tmp4, cnt_core)
            nc.sync.dma_start(out=cc_in[:], in_=stats_l[:])
            # re-warm the ln/exp tables while the collective runs
            nc.scalar.activation(warm, warm, AF.Ln)
            nc.scalar.activation(warm, warm, AF.Exp)
            nc.gpsimd.collective_compute(
                "AllReduce", ALU.add,
                replica_groups=[list(range(NCORES))],
                ins=[cc_in[:].opt()], outs=[cc_out[:].opt()],
            )
            stats_g = persist.tile([P, 2 * CT2], f32, tag="stats_g")
            nc.sync.dma_start(out=stats_g[:], in_=cc_out[:])

            mom = persist.tile([P, 2 * CT2], f32, tag="mom")
            nc.vector.tensor_scalar_mul(mom, stats_g, 1.0 / cnt_all)
            var = persist.tile([P, CT2], f32, tag="var")
            nc.vector.tensor_mul(var, mom[:, 0:CT2], mom[:, 0:CT2])
            nc.vector.tensor_sub(var, mom[:, CT2:], var)
            # rsqrt = exp(-0.5 ln(var+eps)); same act table set as the exp
            lnv = persist.tile([P, CT2], f32, tag="lnv")
            nc.scalar.activation(lnv, var, AF.Ln, bias=eps_t)
            inv = persist.tile([P, CT2], f32, tag="inv")
            nc.scalar.activation(inv, lnv, AF.Exp, scale=-0.5)
            scl4 = persist.tile([P, CT2], f32, tag="scl4")
            nc.vector.tensor_mul(scl4, gamma4, inv)
            rscl = persist.tile([P, CT2], f32, tag="rscl")
            nc.vector.reciprocal(rscl, scl4)
            t4 = persist.tile([P, CT2], f32, tag="t4")
            nc.vector.tensor_mul(t4, beta4, rscl)
            nc.vector.tensor_sub(t4, t4, mom[:, 0:CT2])
            # W2' = W2 * scl (per input channel = partition x ktile)
            w2x = persist.tile([P, CT2, D], bf16, tag="w2x")
            for k in range(CT2):
                nc.vector.tensor_scalar_mul(w2x[:, k, :], w2s[:, k, :], scl4[:, k:k + 1])

            # ================= pass 2 =================
            for p in range(PAIRS):
                for j in range(NCH):
                    sl = slice(j * CHUNK, (j + 1) * CHUNK)
                    h1n = work.tile([P, CT2, CHUNK], bf16, tag="h1n", name="h1n")
                    for m in range(CT2):
                        if m < 2:
                            # relu(h + t) fused on the scalar engine
                            nc.scalar.activation(
                                h1n[:, m, :], h1[p][:, m, sl], AF.Relu,
                                bias=t4[:, m:m + 1],
                            )
                        else:
                            nc.vector.tensor_scalar_add(
                                h1n[:, m, :], h1[p][:, m, sl], t4[:, m:m + 1]
                            )
                            nc.vector.tensor_scalar_max(
                                h1n[:, m, :], h1n[:, m, :], 0.0
                            )
                    ps = pmmw.tile([P, 2, CHUNK], f32, tag="mmw", name="mmw")
                    for c in range(CT):
                        for k in range(CT2):
                            nc.tensor.matmul(
                                ps[:, c, :],
                                w2x[:, k, c * P:(c + 1) * P],
                                h1n[:, k, :],
                                start=(k == 0), stop=(k == CT2 - 1),
                            )
                    ob = work.tile([P, CT, CHUNK], bf16, tag="ob", name="ob")
                    for c in range(CT):
                        nc.scalar.activation(
                            ob[:, c, :], ps[:, c, :], AF.Identity,
                            bias=b2col[:, c:c + 1],
                        )
                    for c in range(CT):
                        q = nc.sync if (j + c) % 2 == 0 else nc.scalar
                        q.dma_start(out=outd[p, c, :, sl], in_=ob[:, c, :])

    nc.finalize()
    return nc


def _get_nc():
    if "nc" not in _CACHE:
        _CACHE["nc"] = build_bass()
    return _CACHE["nc"]


def _prep_inputs(inputs):
    x = np.asarray(inputs["x"], np.float32)
    source = np.asarray(inputs["source"], np.float32)
    Wq = np.asarray(inputs["Wq"], np.float32)
    Wk = np.asarray(inputs["Wk"], np.float32)
    Wv = np.asarray(inputs["Wv"], np.float32)
    Wm = np.asarray(inputs["Wm"], np.float32)
    W1 = np.asarray(inputs["W1"], np.float32)
    W2 = np.asarray(inputs["W2"], np.float32)
    bq = np.asarray(inputs["bq"], np.float32)
    bk = np.asarray(inputs["bk"], np.float32)

    def to_pairs(a, dt):
        a = a.transpose(0, 2, 1, 3).reshape(B * H, CT, P, N)
        a = np.ascontiguousarray(a.transpose(0, 2, 1, 3))
        if dt is F8:
            a = np.clip(a, -240, 240)
        return a.astype(dt)

    def lhsT(w, dt, scale=1.0):
        wT = np.ascontiguousarray(w.T * scale)
        cin, cout = wT.shape
        a = wT.reshape(cin // P, P, cout).transpose(1, 0, 2)
        a = np.ascontiguousarray(a)
        if dt is F8:
            a = np.clip(a, -240, 240)
        return a.astype(dt)

    def vcol(b):
        return np.asarray(b, np.float32).reshape(-1, P).T

    A = Wq.T @ Wk
    Bm = Wm @ Wv

    # per-key softmax bias term exp((bq . k_m)/16) (zero bq -> ones)
    if np.any(bq):
        kfull = np.einsum("oc,bchn->bohn", Wk, source) + bk[None, :, None, None]
        dm = np.einsum("c,bchn->bhn", bq, kfull) / 16.0
        expd = np.exp(dm).reshape(B * H, MT, P).transpose(0, 2, 1)  # [pair, P, MT]
    else:
        expd = np.ones((B * H, P, MT), np.float32)

    vecs_core = []
    for i in range(NCORES):
        vec = np.zeros((P, 80), np.float32)
        for p in range(PAIRS):
            g = i * PAIRS + p
            vec[:, 16 * p:16 * (p + 1)] = 0.25 * expd[g]
            vec[:, 32 + 16 * p:32 + 16 * (p + 1)] = 32.0 * expd[g]
        vec[:, 64:66] = vcol(inputs["b2"])
        vec[:, 66:70] = vcol(inputs["gamma"])
        vec[:, 70:74] = vcol(inputs["beta"])
        vec[:, 74] = EPS
        vecs_core.append(vec)

    x8 = to_pairs(x, F8)
    x16 = to_pairs(x, BF16)
    s8 = to_pairs(source, F8)

    common = {
        "a8": lhsT(A, F8, SA),
        "b8": lhsT(Bm, F8, SB),
        "w1": lhsT(W1, BF16),
        "w2": lhsT(W2, BF16),
    }
    in_maps = []
    for i in range(NCORES):
        m = dict(common)
        m["vec"] = vecs_core[i]
        m["x8"] = np.ascontiguousarray(x8[i * PAIRS:(i + 1) * PAIRS])
        m["x16"] = np.ascontiguousarray(x16[i * PAIRS:(i + 1) * PAIRS])
        m["s8"] = np.ascontiguousarray(s8[i * PAIRS:(i + 1) * PAIRS])
        in_maps.append(m)
    return in_maps


def run_on_hw(inputs, trace=False, **kw):
    nc = _get_nc()
    in_maps = _prep_inputs(inputs)
    res = run_bass_kernel_spmd(
        nc, in_maps, core_ids=list(range(NCORES)), trace=trace, **kw
    )
    outs = res.results
    full = np.empty((B, H, D, N), np.float32)
    for i in range(NCORES):
        o = np.asarray(outs[i]["out"]).astype(np.float32).reshape(PAIRS, D, N)
        for jp in range(PAIRS):
            g = i * PAIRS + jp
            full[g // H, g % H] = o[jp]
    return full.transpose(0, 2, 1, 3), res


def kernel(**inputs) -> np.ndarray:
    out, _ = run_on_hw(inputs, trace=False)
    return out
